# revision 9
# baseline (speedup 1.0000x reference)
"""Trainium2 Bass kernel for nn_CLIFFHead (moe_routing).

Reference math (B=4096, DZ=2048, DE=128, M=32, H=1024):
    b    = z @ base_W.T + base_b                      [B, 3]
    q    = l2norm(z @ gate_W.T + gate_b)              [B, DE]
    sims = q @ l2norm(E).T                            [B, M]
    h    = gelu(einsum('bi,mhi->bmh', z, W1z) + einsum('me,mhe->mh', E, W1e) + b1)
    delta= einsum('bmh,moh->bmo', h, W2) + b2         [B, M, 3]
    L    = (b[:,None,:] + delta).reshape(B, 3*M)      [B, 96]
    returns (L, sims)

Sharding: expert-parallel across 8 cores — core c owns materials
m = 4c..4c+3, sees the full batch. Gate/base/sims are replicated
(tiny next to the 550-GFLOP grouped GEMM). Host does layout/dtype
transforms only; all FLOPs run on the NeuronCores.

On-chip dataflow per core (all matmuls bf16 with f32 PSUM):
    zT resident per 512-row batch block; W1 shard streamed per
    (expert, h-tile) as pre-transposed [dz,128]x[h,128] tiles;
    hT = gelu(W1z.T-tiles @ zT + bias) stays in SBUF and feeds the
    [3, 512] second matmul, software-pipelined one h-tile behind so
    the TensorEngine never waits on the Scalar engine's gelu.
Outputs "lt" [12, B] and "sims" [B, 4] per core; host transposes /
concatenates.
"""

import sys

for _p in ("/opt/trn_rl_repo",):
    if _p not in sys.path:
        sys.path.insert(0, _p)

import numpy as np
import ml_dtypes

import concourse.bacc as bacc
import concourse.mybir as mybir
import concourse.tile as tile
from concourse.bass_utils import run_bass_kernel_spmd

BF16 = mybir.dt.bfloat16
F32 = mybir.dt.float32
BF = ml_dtypes.bfloat16

N_CORES = 8
B, DZ, DE, M, H = 4096, 2048, 128, 32, 1024
MLOC = M // N_CORES          # 4 experts per core
NBLK = 8                     # batch blocks of 512
BB = B // NBLK               # 512
KD = DZ // 128               # 16 dz k-tiles
KD1 = KD + 1                 # +1 de k-tile (the W1e part of the concat)
HT = H // 128                # 8 h-tiles

ACT = mybir.ActivationFunctionType


def build_nc():
    nc = bacc.Bacc("TRN2", target_bir_lowering=False, debug=False,
                   num_devices=N_CORES)

    zt_d = nc.dram_tensor("zt", [NBLK, 128, KD, BB], BF16, kind="ExternalInput")
    w1t_d = nc.dram_tensor("w1t", [MLOC, HT, 128, KD1, 128], BF16,
                           kind="ExternalInput")
    w2t_d = nc.dram_tensor("w2t", [128, MLOC, HT, 3], BF16, kind="ExternalInput")
    gwt_d = nc.dram_tensor("gwt", [128, KD, DE], BF16, kind="ExternalInput")
    bwt_d = nc.dram_tensor("bwt", [128, KD, 3], BF16, kind="ExternalInput")
    b1c_d = nc.dram_tensor("b1c", [128, MLOC * HT], F32, kind="ExternalInput")
    b2c_d = nc.dram_tensor("b2c", [3, MLOC], F32, kind="ExternalInput")
    bab_d = nc.dram_tensor("bab", [3, 1], F32, kind="ExternalInput")
    gb_d = nc.dram_tensor("gb", [128, 1], F32, kind="ExternalInput")
    eloc_d = nc.dram_tensor("eloc", [MLOC, DE], F32, kind="ExternalInput")
    eye4_d = nc.dram_tensor("eye4", [MLOC, MLOC], BF16, kind="ExternalInput")

    lt_d = nc.dram_tensor("lt", [3 * MLOC, B], F32, kind="ExternalOutput")
    sims_d = nc.dram_tensor("sims", [B, MLOC], F32, kind="ExternalOutput")

    with tile.TileContext(nc) as tc:
        with (
            tc.tile_pool(name="const", bufs=1) as cp,
            tc.tile_pool(name="zt", bufs=2) as ztp,
            tc.tile_pool(name="w1", bufs=4) as w1p,
            tc.tile_pool(name="w1e", bufs=4) as w1ep,
            tc.tile_pool(name="h", bufs=3) as hp,
            tc.tile_pool(name="g", bufs=2) as gp,
            tc.tile_pool(name="st", bufs=3) as stp,
            tc.tile_pool(name="psA", bufs=2, space="PSUM") as psA,
            tc.tile_pool(name="psD", bufs=2, space="PSUM") as psD,
            tc.tile_pool(name="psC", bufs=2, space="PSUM") as psC,
        ):
            # ---- constants
            gwt = cp.tile([128, KD * DE], BF16)
            nc.sync.dma_start(gwt[:], gwt_d[:])
            bwt = cp.tile([128, KD, 3], BF16)
            nc.sync.dma_start(bwt[:], bwt_d[:])
            w2 = cp.tile([128, MLOC * HT * 3], BF16)
            nc.sync.dma_start(w2[:], w2t_d[:])
            b1c = cp.tile([128, MLOC * HT], F32)
            nc.sync.dma_start(b1c[:], b1c_d[:])
            b2c = cp.tile([3, MLOC], F32)
            nc.sync.dma_start(b2c[:], b2c_d[:])
            bab = cp.tile([3, 1], F32)
            nc.sync.dma_start(bab[:], bab_d[:])
            gb = cp.tile([128, 1], F32)
            nc.sync.dma_start(gb[:], gb_d[:])
            eloc = cp.tile([MLOC, DE], F32)
            nc.sync.dma_start(eloc[:], eloc_d[:])
            eye4 = cp.tile([MLOC, MLOC], BF16)
            nc.sync.dma_start(eye4[:], eye4_d[:])
            ones = cp.tile([128, 1], BF16)
            nc.vector.memset(ones[:], 1.0)
            epsE = cp.tile([MLOC, 1], F32)
            nc.vector.memset(epsE[:], 1e-30)
            eps128 = cp.tile([128, 1], F32)
            nc.vector.memset(eps128[:], 1e-30)

            # ---- E prep: En = E / max(||E||row, eps); transposes via PE
            eloc_bf = cp.tile([MLOC, DE], BF16)
            nc.vector.tensor_copy(eloc_bf[:], eloc[:])
            sq4 = cp.tile([MLOC, DE], F32)
            nc.vector.tensor_tensor(sq4[:], eloc[:], eloc[:],
                                    mybir.AluOpType.mult)
            ssE = cp.tile([MLOC, 1], F32)
            nc.vector.reduce_sum(ssE[:], sq4[:], axis=mybir.AxisListType.X)
            nE = cp.tile([MLOC, 1], F32)
            nc.scalar.activation(nE[:], ssE[:], ACT.Sqrt, bias=epsE[:])
            rsE = cp.tile([MLOC, 1], F32)
            nc.vector.reciprocal(rsE[:], nE[:])
            en_bf = cp.tile([MLOC, DE], BF16)
            nc.vector.tensor_scalar_mul(en_bf[:], eloc[:], rsE[:])

            et = cp.tile([128, MLOC], BF16)     # E.T (raw), for emb
            pt = psC.tile([128, MLOC], BF16, tag="t")
            nc.tensor.transpose(pt[:], eloc_bf[:], eye4[:])
            nc.vector.tensor_copy(et[:], pt[:])
            ent = cp.tile([128, MLOC], BF16)    # l2norm(E).T, for sims
            pt2 = psC.tile([128, MLOC], BF16, tag="t")
            nc.tensor.transpose(pt2[:], en_bf[:], eye4[:])
            nc.vector.tensor_copy(ent[:], pt2[:])

            # ---- emb pre-pass: b1eff[:, m*HT+ht] = b1 + einsum('e,he->h', E_m, W1e_m)
            b1eff = cp.tile([128, MLOC * HT], F32)
            for m in range(MLOC):
                for ht in range(HT):
                    w1e = w1ep.tile([128, 128], BF16)
                    nc.sync.dma_start(w1e[:], w1t_d[m, ht, :, KD, :])
                    pe = psC.tile([128, 1], F32, tag="c")
                    nc.tensor.matmul(pe[:], w1e[:], et[:, m:m + 1],
                                     start=True, stop=True)
                    col = m * HT + ht
                    nc.scalar.activation(b1eff[:, col:col + 1], pe[:],
                                         ACT.Identity, bias=b1c[:, col:col + 1])

            # ---- main loop over batch blocks
            for blk in range(NBLK):
                zt = ztp.tile([128, KD, BB], BF16)
                nc.sync.dma_start(zt[:], zt_d[blk])

                # gate logits gT = gate_W @ z.T   [DE, BB]
                pg = psC.tile([128, BB], F32, tag="c")
                for k in range(KD):
                    nc.tensor.matmul(pg[:], gwt[:, k * DE:(k + 1) * DE],
                                     zt[:, k, :], start=(k == 0),
                                     stop=(k == KD - 1))
                # base head bT = base_W @ z.T    [3, BB]
                pb = psC.tile([3, BB], F32, tag="c")
                for k in range(KD):
                    nc.tensor.matmul(pb[:], bwt[:, k, :], zt[:, k, :],
                                     start=(k == 0), stop=(k == KD - 1))

                gT = gp.tile([128, BB], BF16)
                nc.scalar.activation(gT[:], pg[:], ACT.Identity, bias=gb[:])
                gsq = gp.tile([128, BB], BF16, tag="gsq")
                nc.scalar.activation(gsq[:], pg[:], ACT.Square, bias=gb[:])
                bt = gp.tile([3, BB], F32, tag="bt")
                nc.vector.tensor_copy(bt[:], pb[:])

                # per 128-row tile: 1/||g|| then sims = (gT.T @ En.T) * rnorm
                rns = gp.tile([128, 4], F32, tag="rns")
                gn = gp.tile([128, 4], F32, tag="gn")
                for t in range(4):
                    pss = psC.tile([128, 1], F32, tag="c")
                    nc.tensor.matmul(pss[:], gsq[:, t * 128:(t + 1) * 128],
                                     ones[:], start=True, stop=True)
                    nc.scalar.activation(gn[:, t:t + 1], pss[:], ACT.Sqrt,
                                         bias=eps128[:])
                    nc.vector.reciprocal(rns[:, t:t + 1], gn[:, t:t + 1])
                for t in range(4):
                    psm = psC.tile([128, MLOC], F32, tag="c")
                    nc.tensor.matmul(psm[:], gT[:, t * 128:(t + 1) * 128],
                                     ent[:], start=True, stop=True)
                    sst = stp.tile([128, MLOC], F32, tag="sims")
                    nc.vector.tensor_scalar_mul(sst[:], psm[:], rns[:, t:t + 1])
                    nc.sync.dma_start(
                        sims_d[blk * BB + t * 128: blk * BB + (t + 1) * 128, :],
                        sst[:])

                # experts: hT = gelu(W1zT-tiles @ zT + b1eff); delta += W2T @ hT
                # MM2 for h-tile ht issues after MM1 group ht+1 so PE never
                # waits on the gelu.
                for m in range(MLOC):
                    pd = psD.tile([3, BB], F32)
                    pend = None  # (ht, h_tile) awaiting its MM2
                    for ht in range(HT):
                        w1 = w1p.tile([128, KD1 * 128], BF16)
                        nc.sync.dma_start(w1[:], w1t_d[m, ht])
                        pa = psA.tile([128, BB], F32)
                        for k in range(KD):
                            nc.tensor.matmul(pa[:], w1[:, k * 128:(k + 1) * 128],
                                             zt[:, k, :], start=(k == 0),
                                             stop=(k == KD - 1))
                        if pend is not None:
                            pht, ph = pend
                            nc.tensor.matmul(
                                pd[:], w2[:, (m * HT + pht) * 3:(m * HT + pht + 1) * 3],
                                ph[:], start=(pht == 0), stop=False)
                        h = hp.tile([128, BB], BF16)
                        col = m * HT + ht
                        nc.scalar.activation(h[:], pa[:], ACT.Gelu,
                                             bias=b1eff[:, col:col + 1])
                        pend = (ht, h)
                    pht, ph = pend
                    nc.tensor.matmul(
                        pd[:], w2[:, (m * HT + pht) * 3:(m * HT + pht + 1) * 3],
                        ph[:], start=(pht == 0), stop=True)

                    st = stp.tile([3, BB], F32, tag="lt")
                    nc.vector.tensor_tensor(st[:], pd[:], bt[:],
                                            mybir.AluOpType.add)
                    nc.vector.tensor_scalar(st[:], st[:], bab[:],
                                            b2c[:, m:m + 1],
                                            mybir.AluOpType.add,
                                            mybir.AluOpType.add)
                    nc.sync.dma_start(
                        lt_d[3 * m:3 * m + 3, blk * BB:(blk + 1) * BB], st[:])

    nc.compile()
    return nc


def pack_inputs(z, base_W, base_b, gate_W, gate_b, E, W1, b1, W2, b2):
    """Host-side layout/dtype transforms + expert-parallel sharding."""
    z = np.asarray(z, np.float32)
    zt = np.ascontiguousarray(
        z.T.reshape(KD, 128, NBLK, BB).transpose(2, 1, 0, 3)).astype(BF)

    gwt = np.ascontiguousarray(
        np.asarray(gate_W, np.float32).T.reshape(KD, 128, DE)
        .transpose(1, 0, 2)).astype(BF)
    bwt = np.ascontiguousarray(
        np.asarray(base_W, np.float32).T.reshape(KD, 128, 3)
        .transpose(1, 0, 2)).astype(BF)
    bab = np.asarray(base_b, np.float32).reshape(3, 1).copy()
    gb = np.asarray(gate_b, np.float32).reshape(128, 1).copy()
    eye4 = np.eye(MLOC, dtype=BF)

    in_maps = []
    for c in range(N_CORES):
        sl = slice(MLOC * c, MLOC * (c + 1))
        W1c = np.asarray(W1[sl], np.float32)
        w1t = np.ascontiguousarray(
            W1c.reshape(MLOC, HT, 128, KD1, 128)
            .transpose(0, 1, 4, 3, 2)).astype(BF)
        W2c = np.asarray(W2[sl], np.float32)
        w2t = np.ascontiguousarray(
            W2c.reshape(MLOC, 3, HT, 128).transpose(3, 0, 2, 1)).astype(BF)
        b1c = np.ascontiguousarray(
            np.asarray(b1[sl], np.float32).reshape(MLOC, HT, 128)
            .transpose(2, 0, 1).reshape(128, MLOC * HT))
        b2c = np.ascontiguousarray(np.asarray(b2[sl], np.float32).T)
        eloc = np.ascontiguousarray(np.asarray(E[sl], np.float32))
        in_maps.append(dict(zt=zt, w1t=w1t, w2t=w2t, gwt=gwt, bwt=bwt,
                            b1c=b1c, b2c=b2c, bab=bab, gb=gb, eloc=eloc,
                            eye4=eye4))
    return in_maps


_NC_CACHE = []


def get_nc():
    if not _NC_CACHE:
        _NC_CACHE.append(build_nc())
    return _NC_CACHE[0]


def kernel(z, base_W, base_b, gate_W, gate_b, E, W1, b1, W2, b2):
    nc = get_nc()
    in_maps = pack_inputs(z, base_W, base_b, gate_W, gate_b, E, W1, b1, W2, b2)
    res = run_bass_kernel_spmd(nc, in_maps, list(range(N_CORES)), trace=False)
    L = np.concatenate([np.asarray(r["lt"], np.float32).T
                        for r in res.results], axis=1)
    sims = np.concatenate([np.asarray(r["sims"], np.float32)
                           for r in res.results], axis=1)
    return L, sims
